# revision 9
# baseline (speedup 1.0000x reference)
"""Linear-attention Trainium2 Bass kernel (mask-packed, bf16).

Reference computation (per batch b, head h):
    qkv = x @ W^T                         (t, 3072)
    q,k,v -> (h, t, 64)
    k masked rows -> -inf; prepend 4 mem-kv rows
    q = softmax(q * d^-0.5, axis=feature)
    k = softmax(k, axis=sequence)
    ctx = k^T v   (64x64);  out = q @ ctx;  out *= mask

Key optimizations over the naive layout:
  - mask sparsity: masked tokens contribute nothing (k rows get zero
    softmax weight, output rows are zeroed), so the host packs only the
    ~50% active tokens per batch; the kernel runs on the packed sequence
    and the host scatters results back. Padded slots get an exp bias of
    -1e30 so they add 0 to the context sums.
  - bf16 matmuls: fp32/f32r streams the moving operand at half rate on
    the PE (measured ~390ns vs ~215ns per 512-wide matmul), so x/w and
    all matmul operands are bf16 (fp32 PSUM accumulate).
  - q softmax normalization during pass A: denominators via a ones-matmul
    (per-head partition sums), reciprocal on DVE, broadcast back across
    partitions with a tiny K=2 matmul, one full-width multiply. Pass B is
    then pure matmul + copy + DMA with no per-block reciprocal chains.
  - k-softmax denominator = ones-columns appended to v in the context
    matmul; division folded into the context finalize (per-partition).
  - two heads per matmul via block-diagonal packing (128-partition use).
  - PE warmup matmuls during the initial weight DMA keep the HAM clock
    gate from running the first real matmuls at half clock.

Sharding: 8 cores = (batch b in 0..3) x (head-half in 0..1); each core
owns one batch and 8 heads (4 head-pairs). No cross-core communication.
Output is produced transposed ([512 cols, T]) so the pass-B matmul can
stream full-width; the host transposes during the scatter.
"""

import numpy as np

D_MODEL = 1024
N_HEADS = 16
D_HEAD = 64
NMEM = 4
SCALE = D_HEAD ** -0.5
B = 4
L = 4096
NCORES = 8
HPC = 8            # heads per core
NPAIR = HPC // 2   # head-pairs per core
ECOLS = HPC * D_HEAD  # 512 output columns per core
NDB = D_MODEL // 128  # 8 contraction blocks
CH = 512           # tokens per pass-A chunk

_CACHE = {}


def build_nc(n_tb):
    """Per-core Bass program for a packed sequence of n_tb 128-token blocks."""
    import concourse.tile as tile
    from concourse import bacc, mybir

    f32 = mybir.dt.float32
    f32r = mybir.dt.float32r
    bf16 = mybir.dt.bfloat16
    AF = mybir.ActivationFunctionType
    MUL = mybir.AluOpType.mult

    T = n_tb * 128

    nc = bacc.Bacc("TRN2", target_bir_lowering=False, debug=False)

    xT = nc.dram_tensor("xT", (D_MODEL, T), bf16, kind="ExternalInput").ap()
    wqT = nc.dram_tensor("wqT", (D_MODEL, ECOLS), bf16, kind="ExternalInput").ap()
    wkT = nc.dram_tensor("wkT", (D_MODEL, ECOLS), bf16, kind="ExternalInput").ap()
    wvT = nc.dram_tensor("wvT", (D_MODEL, ECOLS), bf16, kind="ExternalInput").ap()
    mkp = nc.dram_tensor("mkp", (NPAIR, NMEM, 128), bf16, kind="ExternalInput").ap()
    mvp = nc.dram_tensor("mvp", (NPAIR, NMEM, 130), bf16, kind="ExternalInput").ap()
    biasm = nc.dram_tensor("biasm", (128, n_tb), f32, kind="ExternalInput").ap()
    outT = nc.dram_tensor("outT", (ECOLS, T), f32, kind="ExternalOutput").ap()

    with tile.TileContext(nc) as tc:
        with (
            tc.tile_pool(name="const", bufs=1) as cpool,
            tc.tile_pool(name="big", bufs=1) as bigpool,
            tc.tile_pool(name="ctxps", bufs=1, space="PSUM") as ctx_pool,
            tc.tile_pool(name="small", bufs=8) as small,
            tc.tile_pool(name="xt", bufs=3) as xt_pool,
            tc.tile_pool(name="ek", bufs=3) as ek_pool,
            tc.tile_pool(name="vv", bufs=3) as vv_pool,
        ):
            # ---- tiny inputs first: mem-kv (also warmup fodder), bias ----
            mk_sb = cpool.tile([NMEM, NPAIR * 128], bf16, name="mk_sb", tag="mk_sb")
            nc.sync.dma_start(
                out=mk_sb.rearrange("n (g d) -> n g d", g=NPAIR),
                in_=mkp.rearrange("g n d -> n g d"),
            )
            mv_sb = cpool.tile([NMEM, NPAIR * 130], bf16, name="mv_sb", tag="mv_sb")
            nc.sync.dma_start(
                out=mv_sb.rearrange("n (g e) -> n g e", g=NPAIR),
                in_=mvp.rearrange("g n e -> n g e"),
            )
            biasm_sb = cpool.tile([128, n_tb], f32, name="biasm_sb", tag="biasm_sb")
            nc.sync.dma_start(out=biasm_sb, in_=biasm)

            # ---- weights ----
            wk_sb = cpool.tile([128, NDB * ECOLS], bf16, name="wk_sb", tag="wk_sb")
            wv_sb = cpool.tile([128, NDB * ECOLS], bf16, name="wv_sb", tag="wv_sb")
            wq_sb = cpool.tile([128, NDB * ECOLS], bf16, name="wq_sb", tag="wq_sb")
            for w_sb, w_dram in ((wk_sb, wkT), (wv_sb, wvT), (wq_sb, wqT)):
                nc.sync.dma_start(
                    out=w_sb.rearrange("p (db e) -> p db e", db=NDB),
                    in_=w_dram.rearrange("(db p) e -> p db e", p=128),
                )

            expmk = cpool.tile([NMEM, NPAIR * 128], bf16, name="expmk", tag="expmk")
            nc.scalar.activation(expmk, mk_sb, AF.Exp)

            # ---- constant for the q-softmax normalization ----
            # onesbd[p, c] = 1 where p and c are in the same 64-row head half:
            # onesbd^T @ expq replicates each head's partition-sum across all
            # 64 partitions of that head, so the softmax denominators come out
            # of one matmul already broadcast for the elementwise divide.
            onesbd = cpool.tile([128, 128], bf16, name="onesbd", tag="onesbd")
            nc.vector.memset(onesbd[0:64, 0:64], 1.0)
            nc.vector.memset(onesbd[64:128, 0:64], 0.0)
            nc.vector.memset(onesbd[0:64, 64:128], 0.0)
            nc.vector.memset(onesbd[64:128, 64:128], 1.0)

            # exp(q*scale)/denom for the whole packed batch, kept resident
            expq = bigpool.tile([128, NPAIR, T], bf16, name="expq", tag="expq")

            # normalized context, block-diagonal per pair
            ctxn = cpool.tile([128, NPAIR * 128], bf16, name="ctxn", tag="ctxn")
            nc.vector.memset(ctxn, 0.0)

            # persistent context accumulators, one bank per pair
            ctx_ps = [
                ctx_pool.tile([128, 130], f32, name=f"ctx_ps{g}", tag=f"ctx{g}")
                for g in range(NPAIR)
            ]
            for g in range(NPAIR):
                nc.tensor.matmul(
                    ctx_ps[g],
                    lhsT=expmk[:, g * 128 : (g + 1) * 128],
                    rhs=mv_sb[:, g * 130 : (g + 1) * 130],
                    start=True,
                    stop=False,
                )

            # ---- pass A: projections, q-normalize, context accumulation ----
            with (
                tc.tile_pool(name="pq", bufs=2, space="PSUM") as pq_pool,
                tc.tile_pool(name="pk", bufs=1, space="PSUM") as pk_pool,
                tc.tile_pool(name="pv", bufs=1, space="PSUM") as pv_pool,
                tc.tile_pool(name="rf", bufs=2) as rf_pool,
            ):
                # PE warmup: junk matmuls on mem-kv while weights stream, so
                # the HAM clock gate is at 8/8 when the real matmuls start.
                wps = pq_pool.tile([128, 512], f32, name="wps", tag="pq")
                for _ in range(16):
                    nc.tensor.matmul(
                        wps, lhsT=mk_sb[:, 0:128], rhs=mk_sb[:, 0:512],
                        start=True, stop=True,
                    )

                for c0 in range(0, T, CH):
                    cw = min(CH, T - c0)
                    tbs = cw // 128
                    xt = xt_pool.tile([128, NDB, cw], bf16, name="xt")
                    nc.sync.dma_start(
                        out=xt,
                        in_=xT.rearrange("(db p) t -> p db t", p=128)[:, :, c0 : c0 + cw],
                    )

                    # q: project, exp, per-head denominators, normalize
                    for g in range(NPAIR):
                        pq = pq_pool.tile([128, cw], f32, name="pq", tag="pq")
                        for db in range(NDB):
                            nc.tensor.matmul(
                                pq,
                                lhsT=wq_sb[:, db * ECOLS + g * 128 : db * ECOLS + (g + 1) * 128],
                                rhs=xt[:, db, :],
                                start=(db == 0),
                                stop=(db == NDB - 1),
                            )
                        eq = expq[:, g, c0 : c0 + cw]
                        nc.scalar.activation(eq, pq, AF.Exp, scale=SCALE)
                        dfull = pq_pool.tile([128, cw], f32, name="dfull", tag="pq")
                        nc.tensor.matmul(
                            dfull, lhsT=onesbd, rhs=eq, start=True, stop=True
                        )
                        rf = rf_pool.tile([128, cw], f32, name="rf")
                        nc.vector.reciprocal(rf, dfull)
                        nc.vector.tensor_tensor(eq, eq, rf, MUL)

                    # k/v projection + exp(k)+mask + context accumulation
                    for tbi in range(tbs):
                        j = c0 // 128 + tbi
                        pk = pk_pool.tile([128, ECOLS], f32, name="pk")
                        for db in range(NDB):
                            nc.tensor.matmul(
                                pk,
                                lhsT=xt[:, db, tbi * 128 : (tbi + 1) * 128],
                                rhs=wk_sb[:, db * ECOLS : (db + 1) * ECOLS],
                                start=(db == 0),
                                stop=(db == NDB - 1),
                            )
                        pv = pv_pool.tile([128, ECOLS], f32, name="pv")
                        for db in range(NDB):
                            nc.tensor.matmul(
                                pv,
                                lhsT=xt[:, db, tbi * 128 : (tbi + 1) * 128],
                                rhs=wv_sb[:, db * ECOLS : (db + 1) * ECOLS],
                                start=(db == 0),
                                stop=(db == NDB - 1),
                            )
                        ek = ek_pool.tile([128, ECOLS], bf16, name="ek")
                        nc.scalar.activation(ek, pk, AF.Exp, bias=biasm_sb[:, j : j + 1])
                        vv = vv_pool.tile([128, NPAIR * 130], bf16, name="vv")
                        vv_g = vv.rearrange("p (g e) -> p g e", g=NPAIR)
                        nc.vector.tensor_copy(
                            vv_g[:, :, 0:128],
                            pv.rearrange("p (g e) -> p g e", g=NPAIR),
                        )
                        nc.vector.memset(vv_g[:, :, 128:130], 1.0)
                        for g in range(NPAIR):
                            nc.tensor.matmul(
                                ctx_ps[g],
                                lhsT=ek[:, g * 128 : (g + 1) * 128],
                                rhs=vv[:, g * 130 : (g + 1) * 130],
                                start=False,
                                stop=(j == n_tb - 1),
                            )

            # ---- finalize context + pass B: outT = ctxn^T @ qn ----
            with (
                tc.tile_pool(name="po", bufs=4, space="PSUM") as po_pool,
                tc.tile_pool(name="osb", bufs=4) as osb_pool,
            ):
                for g in range(NPAIR):
                    ps = ctx_ps[g]
                    rk = small.tile([128, 1], f32, name="rk", tag="rk")
                    nc.vector.reciprocal(rk, ps[:, 128:129])
                    o = g * 128
                    nc.vector.tensor_scalar_mul(
                        ctxn[0:64, o : o + 64], ps[0:64, 0:64], rk[0:64]
                    )
                    nc.vector.tensor_scalar_mul(
                        ctxn[64:128, o + 64 : o + 128], ps[64:128, 64:128], rk[64:128]
                    )
                    for ci, c0 in enumerate(range(0, T, CH)):
                        cw = min(CH, T - c0)
                        po = po_pool.tile([128, cw], f32, name="po")
                        nc.tensor.matmul(
                            po,
                            lhsT=ctxn[:, o : o + 128],
                            rhs=expq[:, g, c0 : c0 + cw],
                            start=True,
                            stop=True,
                        )
                        osb = osb_pool.tile([128, cw], f32, name="osb")
                        if (g + ci) % 2 == 0:
                            nc.scalar.activation(osb, po, AF.Copy)
                        else:
                            nc.vector.tensor_copy(osb, po)
                        nc.sync.dma_start(out=outT[o : o + 128, c0 : c0 + cw], in_=osb)

    nc.compile()
    return nc


def _host_inputs(x, w_qkv, mem_kv, mask):
    """Pack active tokens per batch; build the 8 per-core input maps."""
    import ml_dtypes

    bf = ml_dtypes.bfloat16
    x = np.asarray(x, dtype=np.float32)
    w_qkv = np.asarray(w_qkv, dtype=np.float32)
    mem_kv = np.asarray(mem_kv, dtype=np.float32)
    mask = np.asarray(mask)

    idxs = [np.flatnonzero(mask[b]) for b in range(B)]
    n_tb = max(1, max((len(i) + 127) // 128 for i in idxs))
    T = n_tb * 128

    w4 = w_qkv.reshape(N_HEADS, D_HEAD, 3, D_MODEL)
    wT = {}
    for half in (0, 1):
        h0 = half * HPC
        for ci, cn in ((0, "q"), (1, "k"), (2, "v")):
            wT[(half, cn)] = (
                w4[h0 : h0 + HPC, :, ci, :].reshape(ECOLS, D_MODEL).T.astype(bf)
            )

    xTp = []
    biases = []
    for b in range(B):
        idx = idxs[b]
        n = len(idx)
        xp = np.zeros((T, D_MODEL), np.float32)
        if n:
            xp[:n] = x[b][idx]
        xTp.append(xp.T.astype(bf))
        bias = np.zeros(T, np.float32)
        bias[n:] = -1e30
        biases.append(np.ascontiguousarray(bias.reshape(n_tb, 128).T))

    in_maps = []
    for c in range(NCORES):
        b, half = divmod(c, 2)
        h0 = half * HPC
        mk = (
            mem_kv[0, h0 : h0 + HPC]
            .reshape(NPAIR, 2, NMEM, D_HEAD)
            .transpose(0, 2, 1, 3)
            .reshape(NPAIR, NMEM, 128)
        )
        mv = (
            mem_kv[1, h0 : h0 + HPC]
            .reshape(NPAIR, 2, NMEM, D_HEAD)
            .transpose(0, 2, 1, 3)
            .reshape(NPAIR, NMEM, 128)
        )
        mvp = np.ones((NPAIR, NMEM, 130), np.float32)
        mvp[:, :, :128] = mv
        in_maps.append(
            {
                "xT": xTp[b],
                "wqT": wT[(half, "q")],
                "wkT": wT[(half, "k")],
                "wvT": wT[(half, "v")],
                "mkp": np.ascontiguousarray(mk).astype(bf),
                "mvp": mvp.astype(bf),
                "biasm": biases[b],
            }
        )
    return in_maps, idxs, n_tb


def _get_nc(n_tb):
    key = ("nc", n_tb)
    if key not in _CACHE:
        _CACHE[key] = build_nc(n_tb)
    return _CACHE[key]


def kernel(x, w_qkv, mem_kv, mask):
    from concourse.bass_utils import run_bass_kernel_spmd

    in_maps, idxs, n_tb = _host_inputs(x, w_qkv, mem_kv, mask)
    nc = _get_nc(n_tb)
    res = run_bass_kernel_spmd(nc, in_maps, core_ids=list(range(NCORES)))
    out = np.zeros((B, L, D_MODEL), np.float32)
    for c in range(NCORES):
        b, half = divmod(c, 2)
        n = len(idxs[b])
        if n:
            r = res.results[c]["outT"]  # [ECOLS, T]
            out[b][idxs[b], half * ECOLS : (half + 1) * ECOLS] = r[:, :n].T
    return out


# revision 11
# speedup vs baseline: 1.1453x; 1.1453x over previous
"""Linear-attention Trainium2 Bass kernel (mask-packed, bf16).

Reference computation (per batch b, head h):
    qkv = x @ W^T                         (t, 3072)
    q,k,v -> (h, t, 64)
    k masked rows -> -inf; prepend 4 mem-kv rows
    q = softmax(q * d^-0.5, axis=feature)
    k = softmax(k, axis=sequence)
    ctx = k^T v   (64x64);  out = q @ ctx;  out *= mask

Key optimizations over the naive layout:
  - mask sparsity: masked tokens contribute nothing (k rows get zero
    softmax weight, output rows are zeroed), so the host packs only the
    ~50% active tokens per batch; the kernel runs on the packed sequence
    and the host scatters results back. Padded slots get an exp bias of
    -1e30 so they add 0 to the context sums.
  - bf16 matmuls: fp32/f32r streams the moving operand at half rate on
    the PE (measured ~390ns vs ~215ns per 512-wide matmul), so x/w and
    all matmul operands are bf16 (fp32 PSUM accumulate).
  - q softmax normalization during pass A: denominators via a ones-matmul
    (per-head partition sums), reciprocal on DVE, broadcast back across
    partitions with a tiny K=2 matmul, one full-width multiply. Pass B is
    then pure matmul + copy + DMA with no per-block reciprocal chains.
  - k-softmax denominator = ones-columns appended to v in the context
    matmul; division folded into the context finalize (per-partition).
  - two heads per matmul via block-diagonal packing (128-partition use).
  - PE warmup matmuls during the initial weight DMA keep the HAM clock
    gate from running the first real matmuls at half clock.

Sharding: 8 cores = (batch b in 0..3) x (head-half in 0..1); each core
owns one batch and 8 heads (4 head-pairs). No cross-core communication.
Output is produced transposed ([512 cols, T]) so the pass-B matmul can
stream full-width; the host transposes during the scatter.
"""

import numpy as np

D_MODEL = 1024
N_HEADS = 16
D_HEAD = 64
NMEM = 4
SCALE = D_HEAD ** -0.5
B = 4
L = 4096
NCORES = 8
HPC = 8            # heads per core
NPAIR = HPC // 2   # head-pairs per core
ECOLS = HPC * D_HEAD  # 512 output columns per core
NDB = D_MODEL // 128  # 8 contraction blocks
CH = 512           # tokens per pass-A chunk

_CACHE = {}


def build_nc(n_tb):
    """Per-core Bass program for a packed sequence of n_tb 128-token blocks."""
    import concourse.tile as tile
    from concourse import bacc, mybir

    f32 = mybir.dt.float32
    f32r = mybir.dt.float32r
    bf16 = mybir.dt.bfloat16
    AF = mybir.ActivationFunctionType
    MUL = mybir.AluOpType.mult

    T = n_tb * 128

    nc = bacc.Bacc("TRN2", target_bir_lowering=False, debug=False)

    xT = nc.dram_tensor("xT", (D_MODEL, T), bf16, kind="ExternalInput").ap()
    wqT = nc.dram_tensor("wqT", (D_MODEL, ECOLS), bf16, kind="ExternalInput").ap()
    wkT = nc.dram_tensor("wkT", (D_MODEL, ECOLS), bf16, kind="ExternalInput").ap()
    wvT = nc.dram_tensor("wvT", (D_MODEL, ECOLS), bf16, kind="ExternalInput").ap()
    mkp = nc.dram_tensor("mkp", (NPAIR, NMEM, 128), bf16, kind="ExternalInput").ap()
    mvp = nc.dram_tensor("mvp", (NPAIR, NMEM, 130), bf16, kind="ExternalInput").ap()
    biasm = nc.dram_tensor("biasm", (128, n_tb), f32, kind="ExternalInput").ap()
    outT = nc.dram_tensor("outT", (ECOLS, T), f32, kind="ExternalOutput").ap()

    with tile.TileContext(nc) as tc:
        with (
            tc.tile_pool(name="const", bufs=1) as cpool,
            tc.tile_pool(name="big", bufs=1) as bigpool,
            tc.tile_pool(name="ctxps", bufs=1, space="PSUM") as ctx_pool,
            tc.tile_pool(name="small", bufs=8) as small,
            tc.tile_pool(name="xt", bufs=3) as xt_pool,
            tc.tile_pool(name="ek", bufs=3) as ek_pool,
            tc.tile_pool(name="vv", bufs=3) as vv_pool,
        ):
            # ---- tiny inputs first: mem-kv (also warmup fodder), bias ----
            mk_sb = cpool.tile([NMEM, NPAIR * 128], bf16, name="mk_sb", tag="mk_sb")
            nc.sync.dma_start(
                out=mk_sb.rearrange("n (g d) -> n g d", g=NPAIR),
                in_=mkp.rearrange("g n d -> n g d"),
            )
            mv_sb = cpool.tile([NMEM, NPAIR * 130], bf16, name="mv_sb", tag="mv_sb")
            nc.sync.dma_start(
                out=mv_sb.rearrange("n (g e) -> n g e", g=NPAIR),
                in_=mvp.rearrange("g n e -> n g e"),
            )
            biasm_sb = cpool.tile([128, n_tb], f32, name="biasm_sb", tag="biasm_sb")
            nc.sync.dma_start(out=biasm_sb, in_=biasm)

            # ---- weights ----
            wk_sb = cpool.tile([128, NDB * ECOLS], bf16, name="wk_sb", tag="wk_sb")
            wv_sb = cpool.tile([128, NDB * ECOLS], bf16, name="wv_sb", tag="wv_sb")
            wq_sb = cpool.tile([128, NDB * ECOLS], bf16, name="wq_sb", tag="wq_sb")
            # weights ride the Activation engine's DMA queue so they stream in
            # parallel with the x chunks on the Sync queue
            for w_sb, w_dram in ((wk_sb, wkT), (wv_sb, wvT), (wq_sb, wqT)):
                nc.scalar.dma_start(
                    out=w_sb.rearrange("p (db e) -> p db e", db=NDB),
                    in_=w_dram.rearrange("(db p) e -> p db e", p=128),
                )

            expmk = cpool.tile([NMEM, NPAIR * 128], bf16, name="expmk", tag="expmk")
            nc.scalar.activation(expmk, mk_sb, AF.Exp)

            # ---- constant for the q-softmax normalization ----
            # onesbd[p, c] = 1 where p and c are in the same 64-row head half:
            # onesbd^T @ expq replicates each head's partition-sum across all
            # 64 partitions of that head, so the softmax denominators come out
            # of one matmul already broadcast for the elementwise divide.
            onesbd = cpool.tile([128, 128], bf16, name="onesbd", tag="onesbd")
            nc.vector.memset(onesbd[0:64, 0:64], 1.0)
            nc.vector.memset(onesbd[64:128, 0:64], 0.0)
            nc.vector.memset(onesbd[0:64, 64:128], 0.0)
            nc.vector.memset(onesbd[64:128, 64:128], 1.0)

            # exp(q*scale)/denom for the whole packed batch, kept resident
            expq = bigpool.tile([128, NPAIR, T], bf16, name="expq", tag="expq")

            # normalized context, block-diagonal per pair
            ctxn = cpool.tile([128, NPAIR * 128], bf16, name="ctxn", tag="ctxn")
            nc.vector.memset(ctxn, 0.0)

            # persistent context accumulators, one bank per pair
            ctx_ps = [
                ctx_pool.tile([128, 130], f32, name=f"ctx_ps{g}", tag=f"ctx{g}")
                for g in range(NPAIR)
            ]
            for g in range(NPAIR):
                nc.tensor.matmul(
                    ctx_ps[g],
                    lhsT=expmk[:, g * 128 : (g + 1) * 128],
                    rhs=mv_sb[:, g * 130 : (g + 1) * 130],
                    start=True,
                    stop=False,
                )

            # ---- pass A: projections, q-normalize, context accumulation ----
            with (
                tc.tile_pool(name="pq", bufs=2, space="PSUM") as pq_pool,
                tc.tile_pool(name="pk", bufs=1, space="PSUM") as pk_pool,
                tc.tile_pool(name="pv", bufs=1, space="PSUM") as pv_pool,
                tc.tile_pool(name="rf", bufs=2) as rf_pool,
            ):
                # PE warmup: junk matmuls on mem-kv while weights stream, so
                # the HAM clock gate is at 8/8 when the real matmuls start.
                wps = pq_pool.tile([128, 512], f32, name="wps", tag="pq")
                for _ in range(16):
                    nc.tensor.matmul(
                        wps, lhsT=mk_sb[:, 0:128], rhs=mk_sb[:, 0:512],
                        start=True, stop=True,
                    )

                for c0 in range(0, T, CH):
                    cw = min(CH, T - c0)
                    tbs = cw // 128
                    xt = xt_pool.tile([128, NDB, cw], bf16, name="xt")
                    nc.sync.dma_start(
                        out=xt,
                        in_=xT.rearrange("(db p) t -> p db t", p=128)[:, :, c0 : c0 + cw],
                    )

                    # q: project, exp, per-head denominators, normalize
                    for g in range(NPAIR):
                        pq = pq_pool.tile([128, cw], f32, name="pq", tag="pq")
                        for db in range(NDB):
                            nc.tensor.matmul(
                                pq,
                                lhsT=wq_sb[:, db * ECOLS + g * 128 : db * ECOLS + (g + 1) * 128],
                                rhs=xt[:, db, :],
                                start=(db == 0),
                                stop=(db == NDB - 1),
                            )
                        eq = expq[:, g, c0 : c0 + cw]
                        nc.scalar.activation(eq, pq, AF.Exp, scale=SCALE)
                        dfull = pq_pool.tile([128, cw], f32, name="dfull", tag="pq")
                        nc.tensor.matmul(
                            dfull, lhsT=onesbd, rhs=eq, start=True, stop=True
                        )
                        rf = rf_pool.tile([128, cw], f32, name="rf")
                        nc.vector.reciprocal_approx_fast(rf, dfull)
                        nc.vector.tensor_tensor(eq, eq, rf, MUL)

                    # k/v projection + exp(k)+mask + context accumulation
                    for tbi in range(tbs):
                        j = c0 // 128 + tbi
                        pk = pk_pool.tile([128, ECOLS], f32, name="pk")
                        for db in range(NDB):
                            nc.tensor.matmul(
                                pk,
                                lhsT=xt[:, db, tbi * 128 : (tbi + 1) * 128],
                                rhs=wk_sb[:, db * ECOLS : (db + 1) * ECOLS],
                                start=(db == 0),
                                stop=(db == NDB - 1),
                            )
                        pv = pv_pool.tile([128, ECOLS], f32, name="pv")
                        for db in range(NDB):
                            nc.tensor.matmul(
                                pv,
                                lhsT=xt[:, db, tbi * 128 : (tbi + 1) * 128],
                                rhs=wv_sb[:, db * ECOLS : (db + 1) * ECOLS],
                                start=(db == 0),
                                stop=(db == NDB - 1),
                            )
                        ek = ek_pool.tile([128, ECOLS], bf16, name="ek")
                        nc.scalar.activation(ek, pk, AF.Exp, bias=biasm_sb[:, j : j + 1])
                        vv = vv_pool.tile([128, NPAIR * 130], bf16, name="vv")
                        vv_g = vv.rearrange("p (g e) -> p g e", g=NPAIR)
                        nc.vector.tensor_copy(
                            vv_g[:, :, 0:128],
                            pv.rearrange("p (g e) -> p g e", g=NPAIR),
                        )
                        nc.vector.memset(vv_g[:, :, 128:130], 1.0)
                        for g in range(NPAIR):
                            nc.tensor.matmul(
                                ctx_ps[g],
                                lhsT=ek[:, g * 128 : (g + 1) * 128],
                                rhs=vv[:, g * 130 : (g + 1) * 130],
                                start=False,
                                stop=(j == n_tb - 1),
                            )

            # ---- finalize context + pass B: outT = ctxn^T @ qn ----
            with (
                tc.tile_pool(name="po", bufs=4, space="PSUM") as po_pool,
                tc.tile_pool(name="osb", bufs=4) as osb_pool,
            ):
                for g in range(NPAIR):
                    ps = ctx_ps[g]
                    rk = small.tile([128, 1], f32, name="rk", tag="rk")
                    nc.vector.reciprocal(rk, ps[:, 128:129])
                    o = g * 128
                    nc.vector.tensor_scalar_mul(
                        ctxn[0:64, o : o + 64], ps[0:64, 0:64], rk[0:64]
                    )
                    nc.vector.tensor_scalar_mul(
                        ctxn[64:128, o + 64 : o + 128], ps[64:128, 64:128], rk[64:128]
                    )
                    for ci, c0 in enumerate(range(0, T, CH)):
                        cw = min(CH, T - c0)
                        po = po_pool.tile([128, cw], f32, name="po")
                        nc.tensor.matmul(
                            po,
                            lhsT=ctxn[:, o : o + 128],
                            rhs=expq[:, g, c0 : c0 + cw],
                            start=True,
                            stop=True,
                        )
                        osb = osb_pool.tile([128, cw], f32, name="osb")
                        if (g + ci) % 2 == 0:
                            nc.scalar.activation(osb, po, AF.Copy)
                        else:
                            nc.vector.tensor_copy(osb, po)
                        nc.sync.dma_start(out=outT[o : o + 128, c0 : c0 + cw], in_=osb)

    nc.compile()
    return nc


def _host_inputs(x, w_qkv, mem_kv, mask):
    """Pack active tokens per batch; build the 8 per-core input maps."""
    import ml_dtypes

    bf = ml_dtypes.bfloat16
    x = np.asarray(x, dtype=np.float32)
    w_qkv = np.asarray(w_qkv, dtype=np.float32)
    mem_kv = np.asarray(mem_kv, dtype=np.float32)
    mask = np.asarray(mask)

    idxs = [np.flatnonzero(mask[b]) for b in range(B)]
    n_tb = max(1, max((len(i) + 127) // 128 for i in idxs))
    T = n_tb * 128

    w4 = w_qkv.reshape(N_HEADS, D_HEAD, 3, D_MODEL)
    wT = {}
    for half in (0, 1):
        h0 = half * HPC
        for ci, cn in ((0, "q"), (1, "k"), (2, "v")):
            wT[(half, cn)] = (
                w4[h0 : h0 + HPC, :, ci, :].reshape(ECOLS, D_MODEL).T.astype(bf)
            )

    xTp = []
    biases = []
    for b in range(B):
        idx = idxs[b]
        n = len(idx)
        xp = np.zeros((T, D_MODEL), np.float32)
        if n:
            xp[:n] = x[b][idx]
        xTp.append(xp.T.astype(bf))
        bias = np.zeros(T, np.float32)
        bias[n:] = -1e30
        biases.append(np.ascontiguousarray(bias.reshape(n_tb, 128).T))

    in_maps = []
    for c in range(NCORES):
        b, half = divmod(c, 2)
        h0 = half * HPC
        mk = (
            mem_kv[0, h0 : h0 + HPC]
            .reshape(NPAIR, 2, NMEM, D_HEAD)
            .transpose(0, 2, 1, 3)
            .reshape(NPAIR, NMEM, 128)
        )
        mv = (
            mem_kv[1, h0 : h0 + HPC]
            .reshape(NPAIR, 2, NMEM, D_HEAD)
            .transpose(0, 2, 1, 3)
            .reshape(NPAIR, NMEM, 128)
        )
        mvp = np.ones((NPAIR, NMEM, 130), np.float32)
        mvp[:, :, :128] = mv
        in_maps.append(
            {
                "xT": xTp[b],
                "wqT": wT[(half, "q")],
                "wkT": wT[(half, "k")],
                "wvT": wT[(half, "v")],
                "mkp": np.ascontiguousarray(mk).astype(bf),
                "mvp": mvp.astype(bf),
                "biasm": biases[b],
            }
        )
    return in_maps, idxs, n_tb


def _get_nc(n_tb):
    key = ("nc", n_tb)
    if key not in _CACHE:
        _CACHE[key] = build_nc(n_tb)
    return _CACHE[key]


def kernel(x, w_qkv, mem_kv, mask):
    from concourse.bass_utils import run_bass_kernel_spmd

    in_maps, idxs, n_tb = _host_inputs(x, w_qkv, mem_kv, mask)
    nc = _get_nc(n_tb)
    res = run_bass_kernel_spmd(nc, in_maps, core_ids=list(range(NCORES)))
    out = np.zeros((B, L, D_MODEL), np.float32)
    for c in range(NCORES):
        b, half = divmod(c, 2)
        n = len(idxs[b])
        if n:
            r = res.results[c]["outT"]  # [ECOLS, T]
            out[b][idxs[b], half * ECOLS : (half + 1) * ECOLS] = r[:, :n].T
    return out


# revision 21
# speedup vs baseline: 1.2951x; 1.1307x over previous
"""Linear-attention Trainium2 Bass kernel (mask-packed, bf16).

Reference computation (per batch b, head h):
    qkv = x @ W^T                         (t, 3072)
    q,k,v -> (h, t, 64)
    k masked rows -> -inf; prepend 4 mem-kv rows
    q = softmax(q * d^-0.5, axis=feature)
    k = softmax(k, axis=sequence)
    ctx = k^T v   (64x64);  out = q @ ctx;  out *= mask

Key optimizations over the naive layout:
  - mask sparsity: masked tokens contribute nothing (k rows get zero
    softmax weight, output rows are zeroed), so the host packs only the
    ~50% active tokens per batch; the kernel runs on the packed sequence
    and the host scatters results back. Padded slots get an exp bias of
    -1e30 so they add 0 to the context sums.
  - bf16 matmuls: fp32/f32r streams the moving operand at half rate on
    the PE (measured ~390ns vs ~215ns per 512-wide matmul), so x/w and
    all matmul operands are bf16 (fp32 PSUM accumulate).
  - q softmax normalization during pass A: denominators via a ones-matmul
    (per-head partition sums), reciprocal on DVE, broadcast back across
    partitions with a tiny K=2 matmul, one full-width multiply. Pass B is
    then pure matmul + copy + DMA with no per-block reciprocal chains.
  - k-softmax denominator = ones-columns appended to v in the context
    matmul; division folded into the context finalize (per-partition).
  - two heads per matmul via block-diagonal packing (128-partition use).
  - PE warmup matmuls during the initial weight DMA keep the HAM clock
    gate from running the first real matmuls at half clock.

Sharding: 8 cores = (batch b in 0..3) x (head-half in 0..1); each core
owns one batch and 8 heads (4 head-pairs). No cross-core communication.
Output is produced transposed ([512 cols, T]) so the pass-B matmul can
stream full-width; the host transposes during the scatter.
"""

import numpy as np

D_MODEL = 1024
N_HEADS = 16
D_HEAD = 64
NMEM = 4
SCALE = D_HEAD ** -0.5
B = 4
L = 4096
NCORES = 8
HPC = 8            # heads per core
NPAIR = HPC // 2   # head-pairs per core
ECOLS = HPC * D_HEAD  # 512 output columns per core
NDB = D_MODEL // 128  # 8 contraction blocks
CH = 512           # tokens per pass-A chunk

_CACHE = {}


def build_nc(n_tb):
    """Per-core Bass program for a packed sequence of n_tb 128-token blocks."""
    import concourse.tile as tile
    from concourse import bacc, mybir

    f32 = mybir.dt.float32
    f32r = mybir.dt.float32r
    bf16 = mybir.dt.bfloat16
    AF = mybir.ActivationFunctionType
    MUL = mybir.AluOpType.mult

    T = n_tb * 128

    nc = bacc.Bacc("TRN2", target_bir_lowering=False, debug=False)

    f8 = mybir.dt.float8e4
    DR = mybir.MatmulPerfMode.DoubleRow

    xT = nc.dram_tensor("xT", (D_MODEL, T), bf16, kind="ExternalInput").ap()
    x8T = nc.dram_tensor("x8T", (D_MODEL, T), f8, kind="ExternalInput").ap()
    wq8T = nc.dram_tensor("wq8T", (D_MODEL, ECOLS), f8, kind="ExternalInput").ap()
    wkT = nc.dram_tensor("wkT", (D_MODEL, ECOLS), bf16, kind="ExternalInput").ap()
    wvT = nc.dram_tensor("wvT", (D_MODEL, ECOLS), bf16, kind="ExternalInput").ap()
    mkp = nc.dram_tensor("mkp", (NPAIR, NMEM, 128), bf16, kind="ExternalInput").ap()
    mvp = nc.dram_tensor("mvp", (NPAIR, NMEM, 130), bf16, kind="ExternalInput").ap()
    biasm = nc.dram_tensor("biasm", (128, n_tb), f32, kind="ExternalInput").ap()
    outT = nc.dram_tensor("outT", (ECOLS, T), bf16, kind="ExternalOutput").ap()

    with tile.TileContext(nc) as tc:
        with (
            tc.tile_pool(name="const", bufs=1) as cpool,
            tc.tile_pool(name="big", bufs=1) as bigpool,
            tc.tile_pool(name="ctxps", bufs=1, space="PSUM") as ctx_pool,
            tc.tile_pool(name="small", bufs=8) as small,
            tc.tile_pool(name="xt", bufs=3) as xt_pool,
            tc.tile_pool(name="ek", bufs=3) as ek_pool,
            tc.tile_pool(name="vv", bufs=3) as vv_pool,
        ):
            # ---- tiny inputs first: mem-kv (also warmup fodder), bias ----
            mk_sb = cpool.tile([NMEM, NPAIR * 128], bf16, name="mk_sb", tag="mk_sb")
            nc.sync.dma_start(
                out=mk_sb.rearrange("n (g d) -> n g d", g=NPAIR),
                in_=mkp.rearrange("g n d -> n g d"),
            )
            mv_sb = cpool.tile([NMEM, NPAIR * 130], bf16, name="mv_sb", tag="mv_sb")
            nc.sync.dma_start(
                out=mv_sb.rearrange("n (g e) -> n g e", g=NPAIR),
                in_=mvp.rearrange("g n e -> n g e"),
            )
            biasm_sb = cpool.tile([128, n_tb], f32, name="biasm_sb", tag="biasm_sb")
            nc.sync.dma_start(out=biasm_sb, in_=biasm)

            # ---- weights ----
            wk_sb = cpool.tile([128, NDB * ECOLS], bf16, name="wk_sb", tag="wk_sb")
            wv_sb = cpool.tile([128, NDB * ECOLS], bf16, name="wv_sb", tag="wv_sb")
            # weights ride the Activation engine's DMA queue so they stream in
            # parallel with the x chunks on the Sync queue
            for w_sb, w_dram in ((wk_sb, wkT), (wv_sb, wvT)):
                nc.scalar.dma_start(
                    out=w_sb.rearrange("p (db e) -> p db e", db=NDB),
                    in_=w_dram.rearrange("(db p) e -> p db e", p=128),
                )
            wq8_sb = cpool.tile([128, NDB * ECOLS], f8, name="wq8_sb", tag="wq8_sb")
            nc.scalar.dma_start(
                out=wq8_sb.rearrange("p (db e) -> p db e", db=NDB),
                in_=wq8T.rearrange("(db p) e -> p db e", p=128),
            )
            wq8_3 = wq8_sb.rearrange("p (db e) -> p db e", db=NDB)

            expmk = cpool.tile([NMEM, NPAIR * 128], bf16, name="expmk", tag="expmk")
            nc.scalar.activation(expmk, mk_sb, AF.Exp)

            # ---- constant for the q-softmax normalization ----
            # onesbd[p, c] = 1 where p and c are in the same 64-row head half:
            # onesbd^T @ expq replicates each head's partition-sum across all
            # 64 partitions of that head, so the softmax denominators come out
            # of one matmul already broadcast for the elementwise divide.
            onesbd = cpool.tile([128, 128], bf16, name="onesbd", tag="onesbd")
            nc.vector.memset(onesbd[0:64, 0:64], 1.0)
            nc.vector.memset(onesbd[64:128, 0:64], 0.0)
            nc.vector.memset(onesbd[0:64, 64:128], 0.0)
            nc.vector.memset(onesbd[64:128, 64:128], 1.0)

            # exp(q*scale)/denom for the whole packed batch, kept resident
            expq = bigpool.tile([128, NPAIR, T], bf16, name="expq", tag="expq")

            # normalized context, block-diagonal per pair
            ctxn = cpool.tile([128, NPAIR * 128], bf16, name="ctxn", tag="ctxn")
            nc.vector.memset(ctxn, 0.0)

            # persistent context accumulators, one bank per pair
            ctx_ps = [
                ctx_pool.tile([128, 130], f32, name=f"ctx_ps{g}", tag=f"ctx{g}")
                for g in range(NPAIR)
            ]
            for g in range(NPAIR):
                nc.tensor.matmul(
                    ctx_ps[g],
                    lhsT=expmk[:, g * 128 : (g + 1) * 128],
                    rhs=mv_sb[:, g * 130 : (g + 1) * 130],
                    start=True,
                    stop=False,
                )

            # ---- pass A: projections, q-normalize, context accumulation ----
            with (
                tc.tile_pool(name="pq", bufs=2, space="PSUM") as pq_pool,
                tc.tile_pool(name="pk", bufs=1, space="PSUM") as pk_pool,
                tc.tile_pool(name="pv", bufs=1, space="PSUM") as pv_pool,
                tc.tile_pool(name="rf", bufs=2) as rf_pool,
                tc.tile_pool(name="xt8", bufs=3) as xt8_pool,
            ):
                # PE warmup: junk matmuls on mem-kv while weights stream, so
                # the HAM clock gate is at 8/8 when the real matmuls start.
                wps = pq_pool.tile([128, 512], f32, name="wps", tag="pq")
                for _ in range(16):
                    nc.tensor.matmul(
                        wps, lhsT=mk_sb[:, 0:128], rhs=mk_sb[:, 0:512],
                        start=True, stop=True,
                    )

                for c0 in range(0, T, CH):
                    cw = min(CH, T - c0)
                    tbs = cw // 128
                    xt = xt_pool.tile([128, NDB, cw], bf16, name="xt")
                    nc.sync.dma_start(
                        out=xt,
                        in_=xT.rearrange("(db p) t -> p db t", p=128)[:, :, c0 : c0 + cw],
                    )
                    xt8 = xt8_pool.tile([128, NDB, cw], f8, name="xt8")
                    nc.sync.dma_start(
                        out=xt8,
                        in_=x8T.rearrange("(db p) t -> p db t", p=128)[:, :, c0 : c0 + cw],
                    )

                    # q: project (fp8 DoubleRow, host pre-scaled x*8 / wq*64),
                    # exp (descale folded in), per-head denominators, normalize
                    for g in range(NPAIR):
                        pq = pq_pool.tile([128, cw], f32, name="pq", tag="pq")
                        for db2 in range(0, NDB, 2):
                            nc.tensor.matmul(
                                pq,
                                lhsT=wq8_3[:, db2 : db2 + 2, g * 128 : (g + 1) * 128],
                                rhs=xt8[:, db2 : db2 + 2, :],
                                start=(db2 == 0),
                                stop=(db2 == NDB - 2),
                                perf_mode=DR,
                            )
                        eq = expq[:, g, c0 : c0 + cw]
                        nc.scalar.activation(eq, pq, AF.Exp, scale=SCALE / 512.0)
                        dfull = pq_pool.tile([128, cw], f32, name="dfull", tag="pq")
                        nc.tensor.matmul(
                            dfull, lhsT=onesbd, rhs=eq, start=True, stop=True
                        )
                        rf = rf_pool.tile([128, cw], f32, name="rf")
                        nc.vector.reciprocal_approx_fast(rf, dfull)
                        nc.vector.tensor_tensor(eq, eq, rf, MUL)

                    # k/v projection + exp(k)+mask + context accumulation
                    for tbi in range(tbs):
                        j = c0 // 128 + tbi
                        pk = pk_pool.tile([128, ECOLS], f32, name="pk")
                        for db in range(NDB):
                            nc.tensor.matmul(
                                pk,
                                lhsT=xt[:, db, tbi * 128 : (tbi + 1) * 128],
                                rhs=wk_sb[:, db * ECOLS : (db + 1) * ECOLS],
                                start=(db == 0),
                                stop=(db == NDB - 1),
                            )
                        pv = pv_pool.tile([128, ECOLS], f32, name="pv")
                        for db in range(NDB):
                            nc.tensor.matmul(
                                pv,
                                lhsT=xt[:, db, tbi * 128 : (tbi + 1) * 128],
                                rhs=wv_sb[:, db * ECOLS : (db + 1) * ECOLS],
                                start=(db == 0),
                                stop=(db == NDB - 1),
                            )
                        ek = ek_pool.tile([128, ECOLS], bf16, name="ek")
                        nc.scalar.activation(ek, pk, AF.Exp, bias=biasm_sb[:, j : j + 1])
                        vv = vv_pool.tile([128, NPAIR * 130], bf16, name="vv")
                        vv_g = vv.rearrange("p (g e) -> p g e", g=NPAIR)
                        nc.vector.tensor_copy(
                            vv_g[:, :, 0:128],
                            pv.rearrange("p (g e) -> p g e", g=NPAIR),
                        )
                        nc.vector.memset(vv_g[:, :, 128:130], 1.0)
                        for g in range(NPAIR):
                            nc.tensor.matmul(
                                ctx_ps[g],
                                lhsT=ek[:, g * 128 : (g + 1) * 128],
                                rhs=vv[:, g * 130 : (g + 1) * 130],
                                start=False,
                                stop=(j == n_tb - 1),
                            )

            # ---- finalize context + pass B: outT = ctxn^T @ qn ----
            with (
                tc.tile_pool(name="po", bufs=4, space="PSUM") as po_pool,
                tc.tile_pool(name="osb", bufs=4) as osb_pool,
            ):
                # keep the PE busy across the finalize handoff so HAM stays
                # at full clock for pass B
                kw = po_pool.tile([128, 512], f32, name="kw", tag="po")
                for _ in range(10):
                    nc.tensor.matmul(
                        kw, lhsT=mk_sb[:, 0:128], rhs=mk_sb[:, 0:512],
                        start=True, stop=True,
                    )
                for g in range(NPAIR):
                    ps = ctx_ps[g]
                    rk = small.tile([128, 1], f32, name="rk", tag="rk")
                    nc.vector.reciprocal(rk, ps[:, 128:129])
                    o = g * 128
                    # context normalize on the Activation engine (vector still
                    # drains the last chunk's q-normalize work here)
                    nc.scalar.activation(
                        ctxn[0:64, o : o + 64], ps[0:64, 0:64], AF.Copy,
                        scale=rk[0:64],
                    )
                    nc.scalar.activation(
                        ctxn[64:128, o + 64 : o + 128], ps[64:128, 64:128], AF.Copy,
                        scale=rk[64:128],
                    )
                    for ci, c0 in enumerate(range(0, T, CH)):
                        cw = min(CH, T - c0)
                        po = po_pool.tile([128, cw], f32, name="po", tag="po")
                        nc.tensor.matmul(
                            po,
                            lhsT=ctxn[:, o : o + 128],
                            rhs=expq[:, g, c0 : c0 + cw],
                            start=True,
                            stop=True,
                        )
                        osb = osb_pool.tile([128, cw], bf16, name="osb")
                        if (g + ci) % 2 == 0:
                            nc.scalar.activation(osb, po, AF.Copy)
                        else:
                            nc.vector.tensor_copy(osb, po)
                        eng = nc.sync if (g + ci) % 2 == 0 else nc.scalar
                        eng.dma_start(out=outT[o : o + 128, c0 : c0 + cw], in_=osb)

    nc.compile()
    return nc


def _host_inputs(x, w_qkv, mem_kv, mask):
    """Pack active tokens per batch; build the 8 per-core input maps."""
    import ml_dtypes

    bf = ml_dtypes.bfloat16
    f8 = ml_dtypes.float8_e4m3
    x = np.asarray(x, dtype=np.float32)
    w_qkv = np.asarray(w_qkv, dtype=np.float32)
    mem_kv = np.asarray(mem_kv, dtype=np.float32)
    mask = np.asarray(mask)

    idxs = [np.flatnonzero(mask[b]) for b in range(B)]
    n_tb = max(1, max((len(i) + 127) // 128 for i in idxs))
    T = n_tb * 128

    w4 = w_qkv.reshape(N_HEADS, D_HEAD, 3, D_MODEL)
    wT = {}
    for half in (0, 1):
        h0 = half * HPC
        for ci, cn in ((0, "q"), (1, "k"), (2, "v")):
            wc = w4[h0 : h0 + HPC, :, ci, :].reshape(ECOLS, D_MODEL).T
            if cn == "q":
                # fp8 DoubleRow path: pre-scale by 64 so ~N(0, 0.02) weights
                # land in e4m3's normal range; descaled in the exp activation
                wT[(half, cn)] = (wc * 64.0).astype(f8)
            else:
                wT[(half, cn)] = wc.astype(bf)

    xTp = []
    x8Tp = []
    biases = []
    for b in range(B):
        idx = idxs[b]
        n = len(idx)
        xp = np.zeros((T, D_MODEL), np.float32)
        if n:
            xp[:n] = x[b][idx]
        xpT = xp.T
        xTp.append(xpT.astype(bf))
        x8Tp.append((xpT * 8.0).astype(f8))
        bias = np.zeros(T, np.float32)
        bias[n:] = -1e30
        biases.append(np.ascontiguousarray(bias.reshape(n_tb, 128).T))

    in_maps = []
    for c in range(NCORES):
        b, half = divmod(c, 2)
        h0 = half * HPC
        mk = (
            mem_kv[0, h0 : h0 + HPC]
            .reshape(NPAIR, 2, NMEM, D_HEAD)
            .transpose(0, 2, 1, 3)
            .reshape(NPAIR, NMEM, 128)
        )
        mv = (
            mem_kv[1, h0 : h0 + HPC]
            .reshape(NPAIR, 2, NMEM, D_HEAD)
            .transpose(0, 2, 1, 3)
            .reshape(NPAIR, NMEM, 128)
        )
        mvp = np.ones((NPAIR, NMEM, 130), np.float32)
        mvp[:, :, :128] = mv
        in_maps.append(
            {
                "xT": xTp[b],
                "x8T": x8Tp[b],
                "wq8T": wT[(half, "q")],
                "wkT": wT[(half, "k")],
                "wvT": wT[(half, "v")],
                "mkp": np.ascontiguousarray(mk).astype(bf),
                "mvp": mvp.astype(bf),
                "biasm": biases[b],
            }
        )
    return in_maps, idxs, n_tb


def _get_nc(n_tb):
    key = ("nc", n_tb)
    if key not in _CACHE:
        _CACHE[key] = build_nc(n_tb)
    return _CACHE[key]


def kernel(x, w_qkv, mem_kv, mask):
    from concourse.bass_utils import run_bass_kernel_spmd

    in_maps, idxs, n_tb = _host_inputs(x, w_qkv, mem_kv, mask)
    nc = _get_nc(n_tb)
    res = run_bass_kernel_spmd(nc, in_maps, core_ids=list(range(NCORES)))
    out = np.zeros((B, L, D_MODEL), np.float32)
    for c in range(NCORES):
        b, half = divmod(c, 2)
        n = len(idxs[b])
        if n:
            r = np.asarray(res.results[c]["outT"], dtype=np.float32)  # [ECOLS, T]
            out[b][idxs[b], half * ECOLS : (half + 1) * ECOLS] = r[:, :n].T
    return out


# revision 26
# speedup vs baseline: 1.3199x; 1.0192x over previous
"""Linear-attention Trainium2 Bass kernel (mask-packed, bf16).

Reference computation (per batch b, head h):
    qkv = x @ W^T                         (t, 3072)
    q,k,v -> (h, t, 64)
    k masked rows -> -inf; prepend 4 mem-kv rows
    q = softmax(q * d^-0.5, axis=feature)
    k = softmax(k, axis=sequence)
    ctx = k^T v   (64x64);  out = q @ ctx;  out *= mask

Key optimizations over the naive layout:
  - mask sparsity: masked tokens contribute nothing (k rows get zero
    softmax weight, output rows are zeroed), so the host packs only the
    ~50% active tokens per batch; the kernel runs on the packed sequence
    and the host scatters results back. Padded slots get an exp bias of
    -1e30 so they add 0 to the context sums.
  - bf16 matmuls: fp32/f32r streams the moving operand at half rate on
    the PE (measured ~390ns vs ~215ns per 512-wide matmul), so x/w and
    all matmul operands are bf16 (fp32 PSUM accumulate).
  - q softmax normalization during pass A: denominators via a ones-matmul
    (per-head partition sums), reciprocal on DVE, broadcast back across
    partitions with a tiny K=2 matmul, one full-width multiply. Pass B is
    then pure matmul + copy + DMA with no per-block reciprocal chains.
  - k-softmax denominator = ones-columns appended to v in the context
    matmul; division folded into the context finalize (per-partition).
  - two heads per matmul via block-diagonal packing (128-partition use).
  - PE warmup matmuls during the initial weight DMA keep the HAM clock
    gate from running the first real matmuls at half clock.

Sharding: 8 cores = (batch b in 0..3) x (head-half in 0..1); each core
owns one batch and 8 heads (4 head-pairs). No cross-core communication.
Output is produced transposed ([512 cols, T]) so the pass-B matmul can
stream full-width; the host transposes during the scatter.
"""

import numpy as np

D_MODEL = 1024
N_HEADS = 16
D_HEAD = 64
NMEM = 4
SCALE = D_HEAD ** -0.5
B = 4
L = 4096
NCORES = 8
HPC = 8            # heads per core
NPAIR = HPC // 2   # head-pairs per core
ECOLS = HPC * D_HEAD  # 512 output columns per core
NDB = D_MODEL // 128  # 8 contraction blocks
CH = 512           # tokens per pass-A chunk

_CACHE = {}


def build_nc(n_tb):
    """Per-core Bass program for a packed sequence of n_tb 128-token blocks."""
    import concourse.tile as tile
    from concourse import bacc, mybir

    f32 = mybir.dt.float32
    f32r = mybir.dt.float32r
    bf16 = mybir.dt.bfloat16
    AF = mybir.ActivationFunctionType
    MUL = mybir.AluOpType.mult

    T = n_tb * 128

    nc = bacc.Bacc("TRN2", target_bir_lowering=False, debug=False)

    f8 = mybir.dt.float8e4
    DR = mybir.MatmulPerfMode.DoubleRow

    xT = nc.dram_tensor("xT", (D_MODEL, T), bf16, kind="ExternalInput").ap()
    x8T = nc.dram_tensor("x8T", (D_MODEL, T), f8, kind="ExternalInput").ap()
    wq8T = nc.dram_tensor("wq8T", (D_MODEL, ECOLS), f8, kind="ExternalInput").ap()
    wkT = nc.dram_tensor("wkT", (D_MODEL, ECOLS), bf16, kind="ExternalInput").ap()
    wvT = nc.dram_tensor("wvT", (D_MODEL, ECOLS), bf16, kind="ExternalInput").ap()
    mkp = nc.dram_tensor("mkp", (NPAIR, NMEM, 128), bf16, kind="ExternalInput").ap()
    mvp = nc.dram_tensor("mvp", (NPAIR, NMEM, 130), bf16, kind="ExternalInput").ap()
    biasm = nc.dram_tensor("biasm", (128, n_tb), f32, kind="ExternalInput").ap()
    outT = nc.dram_tensor("outT", (ECOLS, T), bf16, kind="ExternalOutput").ap()

    with tile.TileContext(nc) as tc:
        with (
            tc.tile_pool(name="const", bufs=1) as cpool,
            tc.tile_pool(name="big", bufs=1) as bigpool,
            tc.tile_pool(name="ctxps", bufs=1, space="PSUM") as ctx_pool,
            tc.tile_pool(name="small", bufs=8) as small,
            tc.tile_pool(name="xt", bufs=3) as xt_pool,
            tc.tile_pool(name="ek", bufs=3) as ek_pool,
            tc.tile_pool(name="vv", bufs=3) as vv_pool,
        ):
            # ---- tiny inputs first: mem-kv (also warmup fodder), bias ----
            mk_sb = cpool.tile([NMEM, NPAIR * 128], bf16, name="mk_sb", tag="mk_sb")
            nc.sync.dma_start(
                out=mk_sb.rearrange("n (g d) -> n g d", g=NPAIR),
                in_=mkp.rearrange("g n d -> n g d"),
            )
            mv_sb = cpool.tile([NMEM, NPAIR * 130], bf16, name="mv_sb", tag="mv_sb")
            nc.sync.dma_start(
                out=mv_sb.rearrange("n (g e) -> n g e", g=NPAIR),
                in_=mvp.rearrange("g n e -> n g e"),
            )
            biasm_sb = cpool.tile([128, n_tb], f32, name="biasm_sb", tag="biasm_sb")
            nc.sync.dma_start(out=biasm_sb, in_=biasm)

            # ---- weights ----
            wk_sb = cpool.tile([128, NDB * ECOLS], bf16, name="wk_sb", tag="wk_sb")
            wv_sb = cpool.tile([128, NDB * ECOLS], bf16, name="wv_sb", tag="wv_sb")
            # weights ride the Activation engine's DMA queue so they stream in
            # parallel with the x chunks on the Sync queue
            for w_sb, w_dram in ((wk_sb, wkT), (wv_sb, wvT)):
                nc.scalar.dma_start(
                    out=w_sb.rearrange("p (db e) -> p db e", db=NDB),
                    in_=w_dram.rearrange("(db p) e -> p db e", p=128),
                )
            wq8_sb = cpool.tile([128, NDB * ECOLS], f8, name="wq8_sb", tag="wq8_sb")
            nc.scalar.dma_start(
                out=wq8_sb.rearrange("p (db e) -> p db e", db=NDB),
                in_=wq8T.rearrange("(db p) e -> p db e", p=128),
            )
            wq8_3 = wq8_sb.rearrange("p (db e) -> p db e", db=NDB)

            expmk = cpool.tile([NMEM, NPAIR * 128], bf16, name="expmk", tag="expmk")
            nc.scalar.activation(expmk, mk_sb, AF.Exp)

            # ---- constant for the q-softmax normalization ----
            # onesbd[p, c] = 1 where p and c are in the same 64-row head half:
            # onesbd^T @ expq replicates each head's partition-sum across all
            # 64 partitions of that head, so the softmax denominators come out
            # of one matmul already broadcast for the elementwise divide.
            onesbd = cpool.tile([128, 128], bf16, name="onesbd", tag="onesbd")
            nc.vector.memset(onesbd[0:64, 0:64], 1.0)
            nc.vector.memset(onesbd[64:128, 0:64], 0.0)
            nc.vector.memset(onesbd[0:64, 64:128], 0.0)
            nc.vector.memset(onesbd[64:128, 64:128], 1.0)

            # exp(q*scale)/denom for the whole packed batch, kept resident
            expq = bigpool.tile([128, NPAIR, T], bf16, name="expq", tag="expq")

            # normalized context, block-diagonal per pair
            ctxn = cpool.tile([128, NPAIR * 128], bf16, name="ctxn", tag="ctxn")
            nc.vector.memset(ctxn, 0.0)

            # persistent context accumulators, one bank per pair
            ctx_ps = [
                ctx_pool.tile([128, 130], f32, name=f"ctx_ps{g}", tag=f"ctx{g}")
                for g in range(NPAIR)
            ]
            for g in range(NPAIR):
                nc.tensor.matmul(
                    ctx_ps[g],
                    lhsT=expmk[:, g * 128 : (g + 1) * 128],
                    rhs=mv_sb[:, g * 130 : (g + 1) * 130],
                    start=True,
                    stop=False,
                )

            # ---- pass A: projections, q-normalize, context accumulation ----
            with (
                tc.tile_pool(name="pq", bufs=2, space="PSUM") as pq_pool,
                tc.tile_pool(name="pk", bufs=1, space="PSUM") as pk_pool,
                tc.tile_pool(name="pv", bufs=1, space="PSUM") as pv_pool,
                tc.tile_pool(name="rf", bufs=2) as rf_pool,
                tc.tile_pool(name="xt8", bufs=3) as xt8_pool,
            ):
                # PE warmup: junk matmuls on mem-kv while weights stream, so
                # the HAM clock gate is at 8/8 when the real matmuls start.
                wps = pq_pool.tile([128, 512], f32, name="wps", tag="pq")
                for _ in range(16):
                    nc.tensor.matmul(
                        wps, lhsT=mk_sb[:, 0:128], rhs=mk_sb[:, 0:512],
                        start=True, stop=True,
                    )

                ek_last = None
                for c0 in range(0, T, CH):
                    cw = min(CH, T - c0)
                    tbs = cw // 128
                    xt = xt_pool.tile([128, NDB, cw], bf16, name="xt")
                    xr = xT.rearrange("(db p) t -> p db t", p=128)
                    if c0 == 0:
                        # split the first chunk per token-block so the k/v
                        # matmuls can start after ~512KB instead of 2MB
                        for tbi in range(tbs):
                            s = tbi * 128
                            nc.sync.dma_start(
                                out=xt[:, :, s : s + 128],
                                in_=xr[:, :, c0 + s : c0 + s + 128],
                            )
                    else:
                        nc.sync.dma_start(out=xt, in_=xr[:, :, c0 : c0 + cw])
                    xt8 = xt8_pool.tile([128, NDB, cw], f8, name="xt8")
                    nc.scalar.dma_start(
                        out=xt8,
                        in_=x8T.rearrange("(db p) t -> p db t", p=128)[:, :, c0 : c0 + cw],
                    )

                    # q: project (fp8 DoubleRow, host pre-scaled x*8 / wq*64),
                    # exp (descale folded in), per-head denominators, normalize
                    for g in range(NPAIR):
                        pq = pq_pool.tile([128, cw], f32, name="pq", tag="pq")
                        for db2 in range(0, NDB, 2):
                            nc.tensor.matmul(
                                pq,
                                lhsT=wq8_3[:, db2 : db2 + 2, g * 128 : (g + 1) * 128],
                                rhs=xt8[:, db2 : db2 + 2, :],
                                start=(db2 == 0),
                                stop=(db2 == NDB - 2),
                                perf_mode=DR,
                            )
                        eq = expq[:, g, c0 : c0 + cw]
                        nc.scalar.activation(eq, pq, AF.Exp, scale=SCALE / 512.0)
                        dfull = pq_pool.tile([128, cw], f32, name="dfull", tag="pq")
                        nc.tensor.matmul(
                            dfull, lhsT=onesbd, rhs=eq, start=True, stop=True
                        )
                        rf = rf_pool.tile([128, cw], f32, name="rf")
                        nc.vector.reciprocal_approx_fast(rf, dfull)
                        # multiply on gpsimd: reads only SBUF, keeps the vector
                        # queue clear for the psum-coupled work
                        nc.gpsimd.tensor_tensor(eq, eq, rf, MUL)

                    # k/v projection + exp(k)+mask + context accumulation
                    for tbi in range(tbs):
                        j = c0 // 128 + tbi
                        pk = pk_pool.tile([128, ECOLS], f32, name="pk")
                        for db in range(NDB):
                            nc.tensor.matmul(
                                pk,
                                lhsT=xt[:, db, tbi * 128 : (tbi + 1) * 128],
                                rhs=wk_sb[:, db * ECOLS : (db + 1) * ECOLS],
                                start=(db == 0),
                                stop=(db == NDB - 1),
                            )
                        pv = pv_pool.tile([128, ECOLS], f32, name="pv")
                        for db in range(NDB):
                            nc.tensor.matmul(
                                pv,
                                lhsT=xt[:, db, tbi * 128 : (tbi + 1) * 128],
                                rhs=wv_sb[:, db * ECOLS : (db + 1) * ECOLS],
                                start=(db == 0),
                                stop=(db == NDB - 1),
                            )
                        ek = ek_pool.tile([128, ECOLS], bf16, name="ek")
                        nc.scalar.activation(ek, pk, AF.Exp, bias=biasm_sb[:, j : j + 1])
                        if j == n_tb - 1:
                            ek_last = ek
                        vv = vv_pool.tile([128, NPAIR * 130], bf16, name="vv")
                        vv_g = vv.rearrange("p (g e) -> p g e", g=NPAIR)
                        if j % 2 == 0:
                            nc.vector.tensor_copy(
                                vv_g[:, :, 0:128],
                                pv.rearrange("p (g e) -> p g e", g=NPAIR),
                            )
                        else:
                            nc.scalar.activation(
                                vv_g[:, :, 0:128],
                                pv.rearrange("p (g e) -> p g e", g=NPAIR),
                                AF.Copy,
                            )
                        nc.vector.memset(vv_g[:, :, 128:130], 1.0)
                        for g in range(NPAIR):
                            nc.tensor.matmul(
                                ctx_ps[g],
                                lhsT=ek[:, g * 128 : (g + 1) * 128],
                                rhs=vv[:, g * 130 : (g + 1) * 130],
                                start=False,
                                stop=(j == n_tb - 1),
                            )

            # ---- finalize context + pass B: outT = ctxn^T @ qn ----
            with (
                tc.tile_pool(name="po", bufs=4, space="PSUM") as po_pool,
                tc.tile_pool(name="osb", bufs=4) as osb_pool,
            ):
                # keep the PE busy across the finalize handoff so HAM stays at
                # full clock for pass B; reading the last ek tile pins these
                # after pass A so the scheduler can't hoist them earlier
                kw = po_pool.tile([128, 512], f32, name="kw", tag="po")
                for _ in range(16):
                    nc.tensor.matmul(
                        kw, lhsT=ek_last[:, 0:128], rhs=ek_last[:, 0:512],
                        start=True, stop=True,
                    )
                for g in range(NPAIR):
                    ps = ctx_ps[g]
                    rk = small.tile([128, 1], f32, name="rk", tag="rk")
                    nc.vector.reciprocal(rk, ps[:, 128:129])
                    o = g * 128
                    # context normalize on the Activation engine (vector still
                    # drains the last chunk's q-normalize work here)
                    nc.scalar.activation(
                        ctxn[0:64, o : o + 64], ps[0:64, 0:64], AF.Copy,
                        scale=rk[0:64],
                    )
                    nc.scalar.activation(
                        ctxn[64:128, o + 64 : o + 128], ps[64:128, 64:128], AF.Copy,
                        scale=rk[64:128],
                    )
                    for ci, c0 in enumerate(range(0, T, CH)):
                        cw = min(CH, T - c0)
                        po = po_pool.tile([128, cw], f32, name="po", tag="po")
                        nc.tensor.matmul(
                            po,
                            lhsT=ctxn[:, o : o + 128],
                            rhs=expq[:, g, c0 : c0 + cw],
                            start=True,
                            stop=True,
                        )
                        osb = osb_pool.tile([128, cw], bf16, name="osb")
                        if (g + ci) % 2 == 0:
                            nc.scalar.activation(osb, po, AF.Copy)
                        else:
                            nc.vector.tensor_copy(osb, po)
                        eng = nc.sync if (g + ci) % 2 == 0 else nc.scalar
                        eng.dma_start(out=outT[o : o + 128, c0 : c0 + cw], in_=osb)

    nc.compile()
    return nc


def _host_inputs(x, w_qkv, mem_kv, mask):
    """Pack active tokens per batch; build the 8 per-core input maps."""
    import ml_dtypes

    bf = ml_dtypes.bfloat16
    f8 = ml_dtypes.float8_e4m3
    x = np.asarray(x, dtype=np.float32)
    w_qkv = np.asarray(w_qkv, dtype=np.float32)
    mem_kv = np.asarray(mem_kv, dtype=np.float32)
    mask = np.asarray(mask)

    idxs = [np.flatnonzero(mask[b]) for b in range(B)]
    n_tb = max(1, max((len(i) + 127) // 128 for i in idxs))
    T = n_tb * 128

    w4 = w_qkv.reshape(N_HEADS, D_HEAD, 3, D_MODEL)
    wT = {}
    for half in (0, 1):
        h0 = half * HPC
        for ci, cn in ((0, "q"), (1, "k"), (2, "v")):
            wc = w4[h0 : h0 + HPC, :, ci, :].reshape(ECOLS, D_MODEL).T
            if cn == "q":
                # fp8 DoubleRow path: pre-scale by 64 so ~N(0, 0.02) weights
                # land in e4m3's normal range; descaled in the exp activation
                wT[(half, cn)] = (wc * 64.0).astype(f8)
            else:
                wT[(half, cn)] = wc.astype(bf)

    xTp = []
    x8Tp = []
    biases = []
    for b in range(B):
        idx = idxs[b]
        n = len(idx)
        xp = np.zeros((T, D_MODEL), np.float32)
        if n:
            xp[:n] = x[b][idx]
        xpT = xp.T
        xTp.append(xpT.astype(bf))
        x8Tp.append((xpT * 8.0).astype(f8))
        bias = np.zeros(T, np.float32)
        bias[n:] = -1e30
        biases.append(np.ascontiguousarray(bias.reshape(n_tb, 128).T))

    in_maps = []
    for c in range(NCORES):
        b, half = divmod(c, 2)
        h0 = half * HPC
        mk = (
            mem_kv[0, h0 : h0 + HPC]
            .reshape(NPAIR, 2, NMEM, D_HEAD)
            .transpose(0, 2, 1, 3)
            .reshape(NPAIR, NMEM, 128)
        )
        mv = (
            mem_kv[1, h0 : h0 + HPC]
            .reshape(NPAIR, 2, NMEM, D_HEAD)
            .transpose(0, 2, 1, 3)
            .reshape(NPAIR, NMEM, 128)
        )
        mvp = np.ones((NPAIR, NMEM, 130), np.float32)
        mvp[:, :, :128] = mv
        in_maps.append(
            {
                "xT": xTp[b],
                "x8T": x8Tp[b],
                "wq8T": wT[(half, "q")],
                "wkT": wT[(half, "k")],
                "wvT": wT[(half, "v")],
                "mkp": np.ascontiguousarray(mk).astype(bf),
                "mvp": mvp.astype(bf),
                "biasm": biases[b],
            }
        )
    return in_maps, idxs, n_tb


def _get_nc(n_tb):
    key = ("nc", n_tb)
    if key not in _CACHE:
        _CACHE[key] = build_nc(n_tb)
    return _CACHE[key]


def kernel(x, w_qkv, mem_kv, mask):
    from concourse.bass_utils import run_bass_kernel_spmd

    in_maps, idxs, n_tb = _host_inputs(x, w_qkv, mem_kv, mask)
    nc = _get_nc(n_tb)
    res = run_bass_kernel_spmd(nc, in_maps, core_ids=list(range(NCORES)))
    out = np.zeros((B, L, D_MODEL), np.float32)
    for c in range(NCORES):
        b, half = divmod(c, 2)
        n = len(idxs[b])
        if n:
            r = np.asarray(res.results[c]["outT"], dtype=np.float32)  # [ECOLS, T]
            out[b][idxs[b], half * ECOLS : (half + 1) * ECOLS] = r[:, :n].T
    return out
